# revision 33
# baseline (speedup 1.0000x reference)
"""MoE SwiGLU feed-forward (E=8 experts, top-2 of 8, D=1024, H=2816) on 8 trn2 cores.

Sharding: expert-parallel, one expert per NeuronCore. The router is tiny
(0.3% of FLOPs) and data-dependent, so routing / token dispatch (the
"all-to-all") and the aux-loss reduction run on the host; each core runs
the full gated SwiGLU FFN for the tokens routed to its expert:

    yT = (silu(W1 @ xT) * (W3 @ xT)).T-contracted-with-W2, scaled by gate

All device matmuls are in a transposed layout (tokens on the free axis)
so no on-chip transposes are needed:
  phase 1: h1T[h, c] = sum_d W1T[d, h] * xT[d, c]   (ditto h3T)
           hT = silu(h1T) * h3T                      (ACT + DVE, fp16)
  phase 2: yT[dd, c] = sum_h W2T[h, dd] * hT[h, c], then * gate[c]

Inputs are pre-tiled on the host into DMA-friendly layouts (>=4KB
contiguous per partition) and cast to fp16 (PSUM accumulates fp32).
"""

import numpy as np

E = 8
TOPK = 2
D = 1024
H = 2816
LB_COEF = 0.01
Z_COEF = 0.001
NCORES = 8
P = 128
DN = D // P   # 8
HN = H // P   # 22

_COMPILED = {}


def capacity(max_count):
    """Token capacity: max routed count, padded to a multiple of 4 (8B rows)."""
    return max(128, -(-max_count // 4) * 4)


def _chunks(C):
    """Split the token axis into near-equal pieces of <=512 (PSUM bank limit).

    Balanced pieces beat [512, remainder]: a tiny remainder matmul is bound
    by the ~25ns PE issue floor, so its streaming is nearly free time lost,
    while two ~C/2 matmuls stream every cycle usefully.
    """
    n = -(-C // 512)
    out = []
    off = 0
    for i in range(n):
        sz = -(-(C - off) // (n - i))
        sz = min(C - off, -(-sz // 4) * 4)
        out.append((off, sz))
        off += sz
    return out


def build_bass(C):
    """Build + compile the per-core Bass program for token capacity C."""
    from contextlib import ExitStack

    import concourse.mybir as mybir
    import concourse.tile as tile
    from concourse import bacc

    fp16 = mybir.dt.float16
    f32 = mybir.dt.float32

    nc = bacc.Bacc(
        "TRN2",
        target_bir_lowering=False,
        debug=False,
        enable_asserts=False,
        num_devices=NCORES,
    )

    xt_d = nc.dram_tensor("xt", [P, DN, C], fp16, kind="ExternalInput").ap()
    # w13[hi, k, w, d, m] = w_w[hi*128+m, d*128+k]  (w=0 -> w1, w=1 -> w3)
    w13_d = nc.dram_tensor("w13", [HN, P, 2, DN, P], fp16, kind="ExternalInput").ap()
    # w2t[dd, k, hi, m] = w2[dd*128+m, hi*128+k]
    w2_d = nc.dram_tensor("w2t", [DN, P, HN, P], fp16, kind="ExternalInput").ap()
    g_d = nc.dram_tensor("g", [P, C], f32, kind="ExternalInput").ap()
    yt_d = nc.dram_tensor("yt", [DN, P, C], f32, kind="ExternalOutput").ap()

    chunks = _chunks(C)

    with tile.TileContext(nc) as tc, ExitStack() as ctx:
        const = ctx.enter_context(tc.tile_pool(name="const", bufs=1))
        w13p = ctx.enter_context(tc.tile_pool(name="w13p", bufs=2))
        w2p = ctx.enter_context(tc.tile_pool(name="w2p", bufs=3))
        silp = ctx.enter_context(tc.tile_pool(name="silp", bufs=3))
        outp = ctx.enter_context(tc.tile_pool(name="outp", bufs=3))
        ps1 = ctx.enter_context(tc.tile_pool(name="ps1", bufs=2, space="PSUM"))
        ps3 = ctx.enter_context(tc.tile_pool(name="ps3", bufs=2, space="PSUM"))
        psy = ctx.enter_context(tc.tile_pool(name="psy", bufs=3, space="PSUM"))
        psw = ctx.enter_context(tc.tile_pool(name="psw", bufs=1, space="PSUM"))

        # PE warmup: dummy matmuls on a zeroed SBUF tile while the startup
        # DMAs are in flight, so the HAM clock gate is near/at 8/8 (2.4 GHz)
        # when the real matmul stream begins, instead of paying the ~3.4us
        # cold window at 1.2 GHz. Sized to end about when the first weight
        # slab + x chunks land (~10-12us); longer risks delaying real work
        # on runs where the NEFF preamble itself is slow.
        warm_lhs = const.tile([P, P], fp16, name="warm_lhs")
        nc.vector.memzero(warm_lhs[:])
        warm_psum = psw.tile([P, P], f32, name="warm_psum")
        for _ in range(52):
            nc.tensor.matmul(
                warm_psum[:], warm_lhs[:], warm_lhs[:],
                start=True, stop=True, skip_group_check=True,
            )

        # Startup loads in consumption order, balanced across the sync and
        # scalar DGE queues: the first h1 group reads wt0's w1-half + xt d0
        # first; wt0's w3-half isn't read until the h3 group ~1us later.
        wt0 = w13p.tile([P, 2, DN, P], fp16, name="wt", tag="wt")
        xt_sb = const.tile([P, DN, C], fp16, name="xt_sb")
        nc.sync.dma_start(wt0[:, 0], w13_d[0, :, 0])
        nc.scalar.dma_start(xt_sb[:, 0], xt_d[:, 0])
        nc.sync.dma_start(xt_sb[:, 1], xt_d[:, 1])
        nc.scalar.dma_start(wt0[:, 1], w13_d[0, :, 1])
        for d in range(2, DN):
            eng = nc.scalar if d % 2 == 0 else nc.sync
            eng.dma_start(xt_sb[:, d], xt_d[:, d])
        ht_sb = const.tile([P, HN, C], fp16, name="ht_sb")

        # phase 1: hT = silu(W1T.T @ xT) * (W3T.T @ xT), one 128-row strip of H
        # per iteration; contraction over D in 8 PSUM-accumulated matmuls.
        for hi in range(HN):
            if hi == 0:
                wt = wt0
            else:
                wt = w13p.tile([P, 2, DN, P], fp16, name="wt", tag="wt")
                nc.sync.dma_start(wt[:, 0], w13_d[hi, :, 0])
                nc.sync.dma_start(wt[:, 1], w13_d[hi, :, 1])
            for off, sz in chunks:
                ph1 = ps1.tile([P, sz], f32, name="ph1", tag="ph1")
                for d in range(DN):
                    nc.tensor.matmul(
                        ph1[:],
                        wt[:, 0, d],
                        xt_sb[:, d, off : off + sz],
                        start=(d == 0),
                        stop=(d == DN - 1),
                    )
                ph3 = ps3.tile([P, sz], f32, name="ph3", tag="ph3")
                for d in range(DN):
                    nc.tensor.matmul(
                        ph3[:],
                        wt[:, 1, d],
                        xt_sb[:, d, off : off + sz],
                        start=(d == 0),
                        stop=(d == DN - 1),
                    )
                sig = silp.tile([P, sz], f32, name="sig", tag="sig")
                nc.scalar.activation(
                    sig[:], ph1[:], mybir.ActivationFunctionType.Sigmoid
                )
                sil = silp.tile([P, sz], f32, name="sil", tag="sil")
                nc.vector.tensor_mul(sil[:], sig[:], ph1[:])
                nc.vector.tensor_mul(ht_sb[:, hi, off : off + sz], sil[:], ph3[:])

        # gates are only needed by phase 2; load late so the startup DMAs
        # (first weight slab + xT) get the full HBM bandwidth.
        g_sb = const.tile([P, C], f32, name="g_sb")
        nc.gpsimd.dma_start(g_sb[:], g_d[:])

        # phase 2: yT = W2T.T @ hT (contraction over H in 22 matmuls), * gate
        for dd in range(DN):
            w2t = w2p.tile([P, HN, P], fp16, name="w2t", tag="w2t")
            nc.sync.dma_start(w2t[:], w2_d[dd])
            for off, sz in chunks:
                py = psy.tile([P, sz], f32, name="py", tag="py")
                for hi in range(HN):
                    nc.tensor.matmul(
                        py[:],
                        w2t[:, hi],
                        ht_sb[:, hi, off : off + sz],
                        start=(hi == 0),
                        stop=(hi == HN - 1),
                    )
                yo = outp.tile([P, sz], f32, name="yo", tag="yo")
                nc.vector.tensor_mul(yo[:], py[:], g_sb[:, off : off + sz])
                # outputs ride the scalar DGE queue (idle in phase 2) so they
                # don't queue behind the w2 slab loads on sync
                nc.scalar.dma_start(yt_d[dd, :, off : off + sz], yo[:])

    nc.compile()
    return nc


def _get_compiled(C):
    if C not in _COMPILED:
        _COMPILED[C] = build_bass(C)
    return _COMPILED[C]


def route(xf, router_w):
    """Host router: top-2 indices, top-2 softmax probs, aux loss (fp32 math)."""
    T = xf.shape[0]
    logits = xf @ router_w.T.astype(np.float32)
    ar = np.arange(T)
    i1 = logits.argmax(1)
    masked = logits.copy()
    masked[ar, i1] = -np.inf
    i2 = masked.argmax(1)
    l1 = logits[ar, i1]
    l2 = logits[ar, i2]
    d21 = np.exp(l2 - l1)  # <= 1
    p1 = 1.0 / (1.0 + d21)
    p2 = d21 / (1.0 + d21)

    m = logits.max(1, keepdims=True)
    ex = np.exp(logits - m)
    sumex = ex.sum(1, keepdims=True)
    all_probs = ex / sumex
    lse = m[:, 0] + np.log(sumex[:, 0])
    counts = np.bincount(np.concatenate([i1, i2]), minlength=E).astype(np.float64)
    f = counts / float(T * TOPK)
    pmean = all_probs.astype(np.float64).mean(0)
    aux = np.float32(
        LB_COEF * E * np.sum(f * pmean)
        + Z_COEF * np.mean(lse.astype(np.float64) ** 2)
    )
    return i1, i2, p1, p2, aux


def make_core_inputs(xf, w1, w3, w2, idxs, gates, C):
    in_maps = []
    for e in range(E):
        idx = idxs[e]
        n = len(idx)
        xe = np.zeros((C, D), np.float16)
        xe[:n] = xf[idx]
        xt = np.ascontiguousarray(xe.T.reshape(DN, P, C).transpose(1, 0, 2))

        w13 = np.stack([w1[e], w3[e]])          # [2, H, D] = [w, hi*128+m, d*128+k]
        w13 = w13.reshape(2, HN, P, DN, P)      # [w, hi, m, d, k]
        w13 = np.ascontiguousarray(
            w13.transpose(1, 4, 0, 3, 2), dtype=np.float16
        )                                        # [hi, k, w, d, m]

        w2t = w2[e].reshape(DN, P, HN, P)       # [dd, m, hi, k]
        w2t = np.ascontiguousarray(
            w2t.transpose(0, 3, 2, 1), dtype=np.float16
        )                                        # [dd, k, hi, m]

        gb = np.zeros((C,), np.float32)
        gb[:n] = gates[e]
        g2 = np.ascontiguousarray(np.broadcast_to(gb, (P, C)))

        in_maps.append({"xt": xt, "w13": w13, "w2t": w2t, "g": g2})
    return in_maps


def kernel(x, router_w, w1, w3, w2):
    x = np.asarray(x, dtype=np.float32)
    router_w = np.asarray(router_w, dtype=np.float32)
    w1 = np.asarray(w1, dtype=np.float32)
    w3 = np.asarray(w3, dtype=np.float32)
    w2 = np.asarray(w2, dtype=np.float32)

    B, S, _ = x.shape
    T = B * S
    xf = x.reshape(T, D)

    i1, i2, p1, p2, aux = route(xf, router_w)

    idxs, gates = [], []
    for e in range(E):
        sel1 = i1 == e
        idx = np.nonzero(sel1 | (i2 == e))[0]
        idxs.append(idx)
        gates.append(np.where(sel1, p1, p2)[idx])
    C = capacity(max(len(ix) for ix in idxs))

    in_maps = make_core_inputs(xf, w1, w3, w2, idxs, gates, C)

    from concourse.bass_utils import run_bass_kernel_spmd

    nc = _get_compiled(C)
    res = run_bass_kernel_spmd(nc, in_maps, core_ids=list(range(NCORES)))

    y = np.zeros((T, D), np.float32)
    for e in range(E):
        yt = res.results[e]["yt"]               # [DN, P, C] f32
        ye = np.asarray(yt, np.float32).reshape(D, C).T
        idx = idxs[e]
        y[idx] += ye[: len(idx)]
    return y.reshape(B, S, D), aux


# revision 34
# speedup vs baseline: 1.0405x; 1.0405x over previous
"""MoE SwiGLU feed-forward (E=8 experts, top-2 of 8, D=1024, H=2816) on 8 trn2 cores.

Sharding: expert-parallel, one expert per NeuronCore. The router is tiny
(0.3% of FLOPs) and data-dependent, so routing / token dispatch (the
"all-to-all") and the aux-loss reduction run on the host; each core runs
the full gated SwiGLU FFN for the tokens routed to its expert:

    yT = (silu(W1 @ xT) * (W3 @ xT)).T-contracted-with-W2, scaled by gate

All device matmuls are in a transposed layout (tokens on the free axis)
so no on-chip transposes are needed:
  phase 1: h1T[h, c] = sum_d W1T[d, h] * xT[d, c]   (ditto h3T)
           hT = silu(h1T) * h3T                      (ACT + DVE, fp16)
  phase 2: yT[dd, c] = sum_h W2T[h, dd] * hT[h, c], then * gate[c]

Inputs are pre-tiled on the host into DMA-friendly layouts (>=4KB
contiguous per partition) and cast to fp16 (PSUM accumulates fp32).
"""

import numpy as np

E = 8
TOPK = 2
D = 1024
H = 2816
LB_COEF = 0.01
Z_COEF = 0.001
NCORES = 8
P = 128
DN = D // P   # 8
HN = H // P   # 22

_COMPILED = {}


def capacity(max_count):
    """Token capacity: max routed count, padded to a multiple of 4 (8B rows)."""
    return max(128, -(-max_count // 4) * 4)


def _chunks(C):
    """Split the token axis into near-equal pieces of <=512 (PSUM bank limit).

    Balanced pieces beat [512, remainder]: a tiny remainder matmul is bound
    by the ~25ns PE issue floor, so its streaming is nearly free time lost,
    while two ~C/2 matmuls stream every cycle usefully.
    """
    n = -(-C // 512)
    out = []
    off = 0
    for i in range(n):
        sz = -(-(C - off) // (n - i))
        sz = min(C - off, -(-sz // 4) * 4)
        out.append((off, sz))
        off += sz
    return out


def build_bass(C):
    """Build + compile the per-core Bass program for token capacity C."""
    from contextlib import ExitStack

    import concourse.mybir as mybir
    import concourse.tile as tile
    from concourse import bacc

    fp16 = mybir.dt.float16
    f32 = mybir.dt.float32

    nc = bacc.Bacc(
        "TRN2",
        target_bir_lowering=False,
        debug=False,
        enable_asserts=False,
        num_devices=NCORES,
    )

    xt_d = nc.dram_tensor("xt", [P, DN, C], fp16, kind="ExternalInput").ap()
    # w13[hi, k, w, d, m] = w_w[hi*128+m, d*128+k]  (w=0 -> w1, w=1 -> w3)
    w13_d = nc.dram_tensor("w13", [HN, P, 2, DN, P], fp16, kind="ExternalInput").ap()
    # w2t[dd, k, hi, m] = w2[dd*128+m, hi*128+k]
    w2_d = nc.dram_tensor("w2t", [DN, P, HN, P], fp16, kind="ExternalInput").ap()
    g_d = nc.dram_tensor("g", [P, C], f32, kind="ExternalInput").ap()
    yt_d = nc.dram_tensor("yt", [DN, P, C], f32, kind="ExternalOutput").ap()

    chunks = _chunks(C)

    with tile.TileContext(nc) as tc, ExitStack() as ctx:
        const = ctx.enter_context(tc.tile_pool(name="const", bufs=1))
        w13p = ctx.enter_context(tc.tile_pool(name="w13p", bufs=3))
        w2p = ctx.enter_context(tc.tile_pool(name="w2p", bufs=3))
        silp = ctx.enter_context(tc.tile_pool(name="silp", bufs=3))
        outp = ctx.enter_context(tc.tile_pool(name="outp", bufs=3))
        ps1 = ctx.enter_context(tc.tile_pool(name="ps1", bufs=2, space="PSUM"))
        ps3 = ctx.enter_context(tc.tile_pool(name="ps3", bufs=2, space="PSUM"))
        psy = ctx.enter_context(tc.tile_pool(name="psy", bufs=3, space="PSUM"))
        psw = ctx.enter_context(tc.tile_pool(name="psw", bufs=1, space="PSUM"))

        # PE warmup: dummy matmuls on a zeroed SBUF tile while the startup
        # DMAs are in flight, so the HAM clock gate is near/at 8/8 (2.4 GHz)
        # when the real matmul stream begins, instead of paying the ~3.4us
        # cold window at 1.2 GHz. Sized to end about when the first weight
        # slab + x chunks land (~10-12us); longer risks delaying real work
        # on runs where the NEFF preamble itself is slow.
        warm_lhs = const.tile([P, P], fp16, name="warm_lhs")
        nc.vector.memzero(warm_lhs[:])
        warm_psum = psw.tile([P, P], f32, name="warm_psum")
        for _ in range(52):
            nc.tensor.matmul(
                warm_psum[:], warm_lhs[:], warm_lhs[:],
                start=True, stop=True, skip_group_check=True,
            )

        # Startup loads in consumption order, balanced across the sync and
        # scalar DGE queues: the first h1 group reads wt0's w1-half + xt d0
        # first; wt0's w3-half isn't read until the h3 group ~1us later.
        wt0 = w13p.tile([P, 2, DN, P], fp16, name="wt", tag="wt")
        xt_sb = const.tile([P, DN, C], fp16, name="xt_sb")
        nc.sync.dma_start(wt0[:, 0], w13_d[0, :, 0])
        nc.scalar.dma_start(xt_sb[:, 0], xt_d[:, 0])
        nc.sync.dma_start(xt_sb[:, 1], xt_d[:, 1])
        nc.scalar.dma_start(wt0[:, 1], w13_d[0, :, 1])
        for d in range(2, DN):
            eng = nc.scalar if d % 2 == 0 else nc.sync
            eng.dma_start(xt_sb[:, d], xt_d[:, d])
        ht_sb = const.tile([P, HN, C], fp16, name="ht_sb")

        # phase 1: hT = silu(W1T.T @ xT) * (W3T.T @ xT), one 128-row strip of H
        # per iteration; contraction over D in 8 PSUM-accumulated matmuls.
        for hi in range(HN):
            if hi == 0:
                wt = wt0
            else:
                wt = w13p.tile([P, 2, DN, P], fp16, name="wt", tag="wt")
                nc.sync.dma_start(wt[:, 0], w13_d[hi, :, 0])
                nc.sync.dma_start(wt[:, 1], w13_d[hi, :, 1])
            for off, sz in chunks:
                ph1 = ps1.tile([P, sz], f32, name="ph1", tag="ph1")
                for d in range(DN):
                    nc.tensor.matmul(
                        ph1[:],
                        wt[:, 0, d],
                        xt_sb[:, d, off : off + sz],
                        start=(d == 0),
                        stop=(d == DN - 1),
                    )
                ph3 = ps3.tile([P, sz], f32, name="ph3", tag="ph3")
                for d in range(DN):
                    nc.tensor.matmul(
                        ph3[:],
                        wt[:, 1, d],
                        xt_sb[:, d, off : off + sz],
                        start=(d == 0),
                        stop=(d == DN - 1),
                    )
                sig = silp.tile([P, sz], f32, name="sig", tag="sig")
                nc.scalar.activation(
                    sig[:], ph1[:], mybir.ActivationFunctionType.Sigmoid
                )
                sil = silp.tile([P, sz], f32, name="sil", tag="sil")
                nc.vector.tensor_mul(sil[:], sig[:], ph1[:])
                nc.vector.tensor_mul(ht_sb[:, hi, off : off + sz], sil[:], ph3[:])

        # gates are only needed by phase 2; load late so the startup DMAs
        # (first weight slab + xT) get the full HBM bandwidth.
        g_sb = const.tile([P, C], f32, name="g_sb")
        nc.gpsimd.dma_start(g_sb[:], g_d[:])

        # phase 2: yT = W2T.T @ hT (contraction over H in 22 matmuls), * gate
        for dd in range(DN):
            w2t = w2p.tile([P, HN, P], fp16, name="w2t", tag="w2t")
            nc.sync.dma_start(w2t[:], w2_d[dd])
            for off, sz in chunks:
                py = psy.tile([P, sz], f32, name="py", tag="py")
                for hi in range(HN):
                    nc.tensor.matmul(
                        py[:],
                        w2t[:, hi],
                        ht_sb[:, hi, off : off + sz],
                        start=(hi == 0),
                        stop=(hi == HN - 1),
                    )
                yo = outp.tile([P, sz], f32, name="yo", tag="yo")
                nc.vector.tensor_mul(yo[:], py[:], g_sb[:, off : off + sz])
                # outputs ride the scalar DGE queue (idle in phase 2) so they
                # don't queue behind the w2 slab loads on sync
                nc.scalar.dma_start(yt_d[dd, :, off : off + sz], yo[:])

    nc.compile()
    return nc


def _get_compiled(C):
    if C not in _COMPILED:
        _COMPILED[C] = build_bass(C)
    return _COMPILED[C]


def route(xf, router_w):
    """Host router: top-2 indices, top-2 softmax probs, aux loss (fp32 math)."""
    T = xf.shape[0]
    logits = xf @ router_w.T.astype(np.float32)
    ar = np.arange(T)
    i1 = logits.argmax(1)
    masked = logits.copy()
    masked[ar, i1] = -np.inf
    i2 = masked.argmax(1)
    l1 = logits[ar, i1]
    l2 = logits[ar, i2]
    d21 = np.exp(l2 - l1)  # <= 1
    p1 = 1.0 / (1.0 + d21)
    p2 = d21 / (1.0 + d21)

    m = logits.max(1, keepdims=True)
    ex = np.exp(logits - m)
    sumex = ex.sum(1, keepdims=True)
    all_probs = ex / sumex
    lse = m[:, 0] + np.log(sumex[:, 0])
    counts = np.bincount(np.concatenate([i1, i2]), minlength=E).astype(np.float64)
    f = counts / float(T * TOPK)
    pmean = all_probs.astype(np.float64).mean(0)
    aux = np.float32(
        LB_COEF * E * np.sum(f * pmean)
        + Z_COEF * np.mean(lse.astype(np.float64) ** 2)
    )
    return i1, i2, p1, p2, aux


def make_core_inputs(xf, w1, w3, w2, idxs, gates, C):
    in_maps = []
    for e in range(E):
        idx = idxs[e]
        n = len(idx)
        xe = np.zeros((C, D), np.float16)
        xe[:n] = xf[idx]
        xt = np.ascontiguousarray(xe.T.reshape(DN, P, C).transpose(1, 0, 2))

        w13 = np.stack([w1[e], w3[e]])          # [2, H, D] = [w, hi*128+m, d*128+k]
        w13 = w13.reshape(2, HN, P, DN, P)      # [w, hi, m, d, k]
        w13 = np.ascontiguousarray(
            w13.transpose(1, 4, 0, 3, 2), dtype=np.float16
        )                                        # [hi, k, w, d, m]

        w2t = w2[e].reshape(DN, P, HN, P)       # [dd, m, hi, k]
        w2t = np.ascontiguousarray(
            w2t.transpose(0, 3, 2, 1), dtype=np.float16
        )                                        # [dd, k, hi, m]

        gb = np.zeros((C,), np.float32)
        gb[:n] = gates[e]
        g2 = np.ascontiguousarray(np.broadcast_to(gb, (P, C)))

        in_maps.append({"xt": xt, "w13": w13, "w2t": w2t, "g": g2})
    return in_maps


def kernel(x, router_w, w1, w3, w2):
    x = np.asarray(x, dtype=np.float32)
    router_w = np.asarray(router_w, dtype=np.float32)
    w1 = np.asarray(w1, dtype=np.float32)
    w3 = np.asarray(w3, dtype=np.float32)
    w2 = np.asarray(w2, dtype=np.float32)

    B, S, _ = x.shape
    T = B * S
    xf = x.reshape(T, D)

    i1, i2, p1, p2, aux = route(xf, router_w)

    idxs, gates = [], []
    for e in range(E):
        sel1 = i1 == e
        idx = np.nonzero(sel1 | (i2 == e))[0]
        idxs.append(idx)
        gates.append(np.where(sel1, p1, p2)[idx])
    C = capacity(max(len(ix) for ix in idxs))

    in_maps = make_core_inputs(xf, w1, w3, w2, idxs, gates, C)

    from concourse.bass_utils import run_bass_kernel_spmd

    nc = _get_compiled(C)
    res = run_bass_kernel_spmd(nc, in_maps, core_ids=list(range(NCORES)))

    y = np.zeros((T, D), np.float32)
    for e in range(E):
        yt = res.results[e]["yt"]               # [DN, P, C] f32
        ye = np.asarray(yt, np.float32).reshape(D, C).T
        idx = idxs[e]
        y[idx] += ye[: len(idx)]
    return y.reshape(B, S, D), aux


# revision 36
# speedup vs baseline: 1.0433x; 1.0027x over previous
"""MoE SwiGLU feed-forward (E=8 experts, top-2 of 8, D=1024, H=2816) on 8 trn2 cores.

Sharding: expert-parallel, one expert per NeuronCore. The router is tiny
(0.3% of FLOPs) and data-dependent, so routing / token dispatch (the
"all-to-all") and the aux-loss reduction run on the host; each core runs
the full gated SwiGLU FFN for the tokens routed to its expert:

    yT = (silu(W1 @ xT) * (W3 @ xT)).T-contracted-with-W2, scaled by gate

All device matmuls are in a transposed layout (tokens on the free axis)
so no on-chip transposes are needed:
  phase 1: h1T[h, c] = sum_d W1T[d, h] * xT[d, c]   (ditto h3T)
           hT = silu(h1T) * h3T                      (ACT + DVE, fp16)
  phase 2: yT[dd, c] = sum_h W2T[h, dd] * hT[h, c], then * gate[c]

Inputs are pre-tiled on the host into DMA-friendly layouts (>=4KB
contiguous per partition) and cast to fp16 (PSUM accumulates fp32).
"""

import numpy as np

E = 8
TOPK = 2
D = 1024
H = 2816
LB_COEF = 0.01
Z_COEF = 0.001
NCORES = 8
P = 128
DN = D // P   # 8
HN = H // P   # 22

_COMPILED = {}


def capacity(max_count):
    """Token capacity: max routed count, padded to a multiple of 4 (8B rows)."""
    return max(128, -(-max_count // 4) * 4)


def _chunks(C):
    """Split the token axis into near-equal pieces of <=512 (PSUM bank limit).

    Balanced pieces beat [512, remainder]: a tiny remainder matmul is bound
    by the ~25ns PE issue floor, so its streaming is nearly free time lost,
    while two ~C/2 matmuls stream every cycle usefully.
    """
    n = -(-C // 512)
    out = []
    off = 0
    for i in range(n):
        sz = -(-(C - off) // (n - i))
        sz = min(C - off, -(-sz // 4) * 4)
        out.append((off, sz))
        off += sz
    return out


def build_bass(C):
    """Build + compile the per-core Bass program for token capacity C."""
    from contextlib import ExitStack

    import concourse.mybir as mybir
    import concourse.tile as tile
    from concourse import bacc

    fp16 = mybir.dt.float16
    f32 = mybir.dt.float32

    nc = bacc.Bacc(
        "TRN2",
        target_bir_lowering=False,
        debug=False,
        enable_asserts=False,
        num_devices=NCORES,
    )

    xt_d = nc.dram_tensor("xt", [P, DN, C], fp16, kind="ExternalInput").ap()
    # w13[hi, k, w, d, m] = w_w[hi*128+m, d*128+k]  (w=0 -> w1, w=1 -> w3)
    w13_d = nc.dram_tensor("w13", [HN, P, 2, DN, P], fp16, kind="ExternalInput").ap()
    # w2t[dd, k, hi, m] = w2[dd*128+m, hi*128+k]
    w2_d = nc.dram_tensor("w2t", [DN, P, HN, P], fp16, kind="ExternalInput").ap()
    g_d = nc.dram_tensor("g", [P, C], f32, kind="ExternalInput").ap()
    yt_d = nc.dram_tensor("yt", [DN, P, C], f32, kind="ExternalOutput").ap()

    chunks = _chunks(C)

    with tile.TileContext(nc) as tc, ExitStack() as ctx:
        const = ctx.enter_context(tc.tile_pool(name="const", bufs=1))
        w13p = ctx.enter_context(tc.tile_pool(name="w13p", bufs=3))
        w2p = ctx.enter_context(tc.tile_pool(name="w2p", bufs=3))
        silp = ctx.enter_context(tc.tile_pool(name="silp", bufs=3))
        outp = ctx.enter_context(tc.tile_pool(name="outp", bufs=3))
        ps1 = ctx.enter_context(tc.tile_pool(name="ps1", bufs=2, space="PSUM"))
        ps3 = ctx.enter_context(tc.tile_pool(name="ps3", bufs=2, space="PSUM"))
        psy = ctx.enter_context(tc.tile_pool(name="psy", bufs=3, space="PSUM"))
        psw = ctx.enter_context(tc.tile_pool(name="psw", bufs=1, space="PSUM"))

        # PE warmup: dummy matmuls on a zeroed SBUF tile while the startup
        # DMAs are in flight, so the HAM clock gate is near/at 8/8 (2.4 GHz)
        # when the real matmul stream begins, instead of paying the ~3.4us
        # cold window at 1.2 GHz. Sized to end about when the first weight
        # slab + x chunks land (~10-12us); longer risks delaying real work
        # on runs where the NEFF preamble itself is slow.
        warm_lhs = const.tile([P, P], fp16, name="warm_lhs")
        nc.vector.memzero(warm_lhs[:])
        warm_psum = psw.tile([P, P], f32, name="warm_psum")
        for _ in range(52):
            nc.tensor.matmul(
                warm_psum[:], warm_lhs[:], warm_lhs[:],
                start=True, stop=True, skip_group_check=True,
            )

        # Startup loads in consumption order, balanced across the sync and
        # scalar DGE queues: the first h1 group reads wt0's w1-half + xt d0
        # first; wt0's w3-half isn't read until the h3 group ~1us later.
        wt0 = w13p.tile([P, 2, DN, P], fp16, name="wt", tag="wt")
        xt_sb = const.tile([P, DN, C], fp16, name="xt_sb")
        nc.sync.dma_start(wt0[:, 0], w13_d[0, :, 0])
        nc.scalar.dma_start(xt_sb[:, 0], xt_d[:, 0])
        nc.sync.dma_start(xt_sb[:, 1], xt_d[:, 1])
        nc.scalar.dma_start(wt0[:, 1], w13_d[0, :, 1])
        xt_last_dmas = []
        for d in range(2, DN):
            eng = nc.scalar if d % 2 == 0 else nc.sync
            dma = eng.dma_start(xt_sb[:, d], xt_d[:, d])
            if d >= DN - 2:
                xt_last_dmas.append(dma)
        ht_sb = const.tile([P, HN, C], fp16, name="ht_sb")

        # phase 1: hT = silu(W1T.T @ xT) * (W3T.T @ xT), one 128-row strip of H
        # per iteration; contraction over D in 8 PSUM-accumulated matmuls.
        for hi in range(HN):
            if hi == 0:
                wt = wt0
            else:
                wt = w13p.tile([P, 2, DN, P], fp16, name="wt", tag="wt")
                dma_a = nc.sync.dma_start(wt[:, 0], w13_d[hi, :, 0])
                nc.sync.dma_start(wt[:, 1], w13_d[hi, :, 1])
                if hi == 1:
                    # Defer the whole weight-slab prefetch stream (sync queue,
                    # in-order issue) until xT has fully landed: the slabs'
                    # transfers otherwise fan out to other HW queues and steal
                    # HBM bandwidth from the startup-critical xT load.
                    from concourse.tile import add_dep_helper

                    for xdma in xt_last_dmas:
                        add_dep_helper(
                            dma_a.ins, xdma.ins, sync=True,
                            reason="w13 prefetch waits for xT",
                        )
            for off, sz in chunks:
                ph1 = ps1.tile([P, sz], f32, name="ph1", tag="ph1")
                for d in range(DN):
                    nc.tensor.matmul(
                        ph1[:],
                        wt[:, 0, d],
                        xt_sb[:, d, off : off + sz],
                        start=(d == 0),
                        stop=(d == DN - 1),
                    )
                ph3 = ps3.tile([P, sz], f32, name="ph3", tag="ph3")
                for d in range(DN):
                    nc.tensor.matmul(
                        ph3[:],
                        wt[:, 1, d],
                        xt_sb[:, d, off : off + sz],
                        start=(d == 0),
                        stop=(d == DN - 1),
                    )
                sig = silp.tile([P, sz], f32, name="sig", tag="sig")
                nc.scalar.activation(
                    sig[:], ph1[:], mybir.ActivationFunctionType.Sigmoid
                )
                sil = silp.tile([P, sz], f32, name="sil", tag="sil")
                nc.vector.tensor_mul(sil[:], sig[:], ph1[:])
                nc.vector.tensor_mul(ht_sb[:, hi, off : off + sz], sil[:], ph3[:])

        # gates are only needed by phase 2; load late so the startup DMAs
        # (first weight slab + xT) get the full HBM bandwidth.
        g_sb = const.tile([P, C], f32, name="g_sb")
        nc.gpsimd.dma_start(g_sb[:], g_d[:])

        # phase 2: yT = W2T.T @ hT (contraction over H in 22 matmuls), * gate
        for dd in range(DN):
            w2t = w2p.tile([P, HN, P], fp16, name="w2t", tag="w2t")
            nc.sync.dma_start(w2t[:], w2_d[dd])
            for off, sz in chunks:
                py = psy.tile([P, sz], f32, name="py", tag="py")
                for hi in range(HN):
                    nc.tensor.matmul(
                        py[:],
                        w2t[:, hi],
                        ht_sb[:, hi, off : off + sz],
                        start=(hi == 0),
                        stop=(hi == HN - 1),
                    )
                yo = outp.tile([P, sz], f32, name="yo", tag="yo")
                nc.vector.tensor_mul(yo[:], py[:], g_sb[:, off : off + sz])
                # outputs ride the scalar DGE queue (idle in phase 2) so they
                # don't queue behind the w2 slab loads on sync
                nc.scalar.dma_start(yt_d[dd, :, off : off + sz], yo[:])

    nc.compile()
    return nc


def _get_compiled(C):
    if C not in _COMPILED:
        _COMPILED[C] = build_bass(C)
    return _COMPILED[C]


def route(xf, router_w):
    """Host router: top-2 indices, top-2 softmax probs, aux loss (fp32 math)."""
    T = xf.shape[0]
    logits = xf @ router_w.T.astype(np.float32)
    ar = np.arange(T)
    i1 = logits.argmax(1)
    masked = logits.copy()
    masked[ar, i1] = -np.inf
    i2 = masked.argmax(1)
    l1 = logits[ar, i1]
    l2 = logits[ar, i2]
    d21 = np.exp(l2 - l1)  # <= 1
    p1 = 1.0 / (1.0 + d21)
    p2 = d21 / (1.0 + d21)

    m = logits.max(1, keepdims=True)
    ex = np.exp(logits - m)
    sumex = ex.sum(1, keepdims=True)
    all_probs = ex / sumex
    lse = m[:, 0] + np.log(sumex[:, 0])
    counts = np.bincount(np.concatenate([i1, i2]), minlength=E).astype(np.float64)
    f = counts / float(T * TOPK)
    pmean = all_probs.astype(np.float64).mean(0)
    aux = np.float32(
        LB_COEF * E * np.sum(f * pmean)
        + Z_COEF * np.mean(lse.astype(np.float64) ** 2)
    )
    return i1, i2, p1, p2, aux


def make_core_inputs(xf, w1, w3, w2, idxs, gates, C):
    in_maps = []
    for e in range(E):
        idx = idxs[e]
        n = len(idx)
        xe = np.zeros((C, D), np.float16)
        xe[:n] = xf[idx]
        xt = np.ascontiguousarray(xe.T.reshape(DN, P, C).transpose(1, 0, 2))

        w13 = np.stack([w1[e], w3[e]])          # [2, H, D] = [w, hi*128+m, d*128+k]
        w13 = w13.reshape(2, HN, P, DN, P)      # [w, hi, m, d, k]
        w13 = np.ascontiguousarray(
            w13.transpose(1, 4, 0, 3, 2), dtype=np.float16
        )                                        # [hi, k, w, d, m]

        w2t = w2[e].reshape(DN, P, HN, P)       # [dd, m, hi, k]
        w2t = np.ascontiguousarray(
            w2t.transpose(0, 3, 2, 1), dtype=np.float16
        )                                        # [dd, k, hi, m]

        gb = np.zeros((C,), np.float32)
        gb[:n] = gates[e]
        g2 = np.ascontiguousarray(np.broadcast_to(gb, (P, C)))

        in_maps.append({"xt": xt, "w13": w13, "w2t": w2t, "g": g2})
    return in_maps


def kernel(x, router_w, w1, w3, w2):
    x = np.asarray(x, dtype=np.float32)
    router_w = np.asarray(router_w, dtype=np.float32)
    w1 = np.asarray(w1, dtype=np.float32)
    w3 = np.asarray(w3, dtype=np.float32)
    w2 = np.asarray(w2, dtype=np.float32)

    B, S, _ = x.shape
    T = B * S
    xf = x.reshape(T, D)

    i1, i2, p1, p2, aux = route(xf, router_w)

    idxs, gates = [], []
    for e in range(E):
        sel1 = i1 == e
        idx = np.nonzero(sel1 | (i2 == e))[0]
        idxs.append(idx)
        gates.append(np.where(sel1, p1, p2)[idx])
    C = capacity(max(len(ix) for ix in idxs))

    in_maps = make_core_inputs(xf, w1, w3, w2, idxs, gates, C)

    from concourse.bass_utils import run_bass_kernel_spmd

    nc = _get_compiled(C)
    res = run_bass_kernel_spmd(nc, in_maps, core_ids=list(range(NCORES)))

    y = np.zeros((T, D), np.float32)
    for e in range(E):
        yt = res.results[e]["yt"]               # [DN, P, C] f32
        ye = np.asarray(yt, np.float32).reshape(D, C).T
        idx = idxs[e]
        y[idx] += ye[: len(idx)]
    return y.reshape(B, S, D), aux


# revision 38
# speedup vs baseline: 1.0653x; 1.0210x over previous
"""MoE SwiGLU feed-forward (E=8 experts, top-2 of 8, D=1024, H=2816) on 8 trn2 cores.

Sharding: expert-parallel, one expert per NeuronCore. The router is tiny
(0.3% of FLOPs) and data-dependent, so routing / token dispatch (the
"all-to-all") and the aux-loss reduction run on the host; each core runs
the full gated SwiGLU FFN for the tokens routed to its expert:

    yT = (silu(W1 @ xT) * (W3 @ xT)).T-contracted-with-W2, scaled by gate

All device matmuls are in a transposed layout (tokens on the free axis)
so no on-chip transposes are needed:
  phase 1: h1T[h, c] = sum_d W1T[d, h] * xT[d, c]   (ditto h3T)
           hT = silu(h1T) * h3T                      (ACT + DVE, fp16)
  phase 2: yT[dd, c] = sum_h W2T[h, dd] * hT[h, c], then * gate[c]

Inputs are pre-tiled on the host into DMA-friendly layouts (>=4KB
contiguous per partition) and cast to fp16 (PSUM accumulates fp32).
"""

import numpy as np

E = 8
TOPK = 2
D = 1024
H = 2816
LB_COEF = 0.01
Z_COEF = 0.001
NCORES = 8
P = 128
DN = D // P   # 8
HN = H // P   # 22

_COMPILED = {}


def capacity(max_count):
    """Token capacity: max routed count, padded to a multiple of 4 (8B rows)."""
    return max(128, -(-max_count // 4) * 4)


def _chunks(C):
    """Split the token axis into near-equal pieces of <=512 (PSUM bank limit).

    Balanced pieces beat [512, remainder]: a tiny remainder matmul is bound
    by the ~25ns PE issue floor, so its streaming is nearly free time lost,
    while two ~C/2 matmuls stream every cycle usefully.
    """
    n = -(-C // 512)
    out = []
    off = 0
    for i in range(n):
        sz = -(-(C - off) // (n - i))
        sz = min(C - off, -(-sz // 4) * 4)
        out.append((off, sz))
        off += sz
    return out


def build_bass(C):
    """Build + compile the per-core Bass program for token capacity C."""
    from contextlib import ExitStack

    import concourse.mybir as mybir
    import concourse.tile as tile
    from concourse import bacc

    fp16 = mybir.dt.float16
    f32 = mybir.dt.float32

    nc = bacc.Bacc(
        "TRN2",
        target_bir_lowering=False,
        debug=False,
        enable_asserts=False,
        num_devices=NCORES,
    )

    xt_d = nc.dram_tensor("xt", [P, DN, C], fp16, kind="ExternalInput").ap()
    # w13[hi, k, w, d, m] = w_w[hi*128+m, d*128+k]  (w=0 -> w1, w=1 -> w3)
    w13_d = nc.dram_tensor("w13", [HN, P, 2, DN, P], fp16, kind="ExternalInput").ap()
    # w2t[dd, k, hi, m] = w2[dd*128+m, hi*128+k]
    w2_d = nc.dram_tensor("w2t", [DN, P, HN, P], fp16, kind="ExternalInput").ap()
    g_d = nc.dram_tensor("g", [P, C], f32, kind="ExternalInput").ap()
    yt_d = nc.dram_tensor("yt", [DN, P, C], f32, kind="ExternalOutput").ap()

    chunks = _chunks(C)

    with tile.TileContext(nc) as tc, ExitStack() as ctx:
        const = ctx.enter_context(tc.tile_pool(name="const", bufs=1))
        w13p = ctx.enter_context(tc.tile_pool(name="w13p", bufs=3))
        w2p = ctx.enter_context(tc.tile_pool(name="w2p", bufs=3))
        silp = ctx.enter_context(tc.tile_pool(name="silp", bufs=3))
        outp = ctx.enter_context(tc.tile_pool(name="outp", bufs=3))
        ps1 = ctx.enter_context(tc.tile_pool(name="ps1", bufs=2, space="PSUM"))
        ps3 = ctx.enter_context(tc.tile_pool(name="ps3", bufs=2, space="PSUM"))
        psy = ctx.enter_context(tc.tile_pool(name="psy", bufs=3, space="PSUM"))
        psw = ctx.enter_context(tc.tile_pool(name="psw", bufs=1, space="PSUM"))

        # PE warmup: dummy matmuls on a zeroed SBUF tile while the startup
        # DMAs are in flight, so the HAM clock gate is near/at 8/8 (2.4 GHz)
        # when the real matmul stream begins, instead of paying the ~3.4us
        # cold window at 1.2 GHz. Sized to end about when the first weight
        # slab + x chunks land (~10-12us); longer risks delaying real work
        # on runs where the NEFF preamble itself is slow.
        warm_lhs = const.tile([P, P], fp16, name="warm_lhs")
        nc.vector.memzero(warm_lhs[:])
        warm_psum = psw.tile([P, P], f32, name="warm_psum")
        for _ in range(52):
            nc.tensor.matmul(
                warm_psum[:], warm_lhs[:], warm_lhs[:],
                start=True, stop=True, skip_group_check=True,
            )

        # Startup loads in consumption order, balanced across the sync and
        # scalar DGE queues: the first h1 group reads wt0's w1-half + xt d0
        # first; wt0's w3-half isn't read until the h3 group ~1us later.
        wt0 = w13p.tile([P, 2, DN, P], fp16, name="wt", tag="wt")
        xt_sb = const.tile([P, DN, C], fp16, name="xt_sb")
        nc.sync.dma_start(wt0[:, 0], w13_d[0, :, 0])
        nc.scalar.dma_start(xt_sb[:, 0], xt_d[:, 0])
        nc.sync.dma_start(xt_sb[:, 1], xt_d[:, 1])
        nc.scalar.dma_start(wt0[:, 1], w13_d[0, :, 1])
        for d in range(2, DN):
            eng = nc.scalar if d % 2 == 0 else nc.sync
            eng.dma_start(xt_sb[:, d], xt_d[:, d])
        ht_sb = const.tile([P, HN, C], fp16, name="ht_sb")

        # phase 1: hT = silu(W1T.T @ xT) * (W3T.T @ xT), one 128-row strip of H
        # per iteration; contraction over D in 8 PSUM-accumulated matmuls.
        for hi in range(HN):
            if hi == 0:
                wt = wt0
            else:
                wt = w13p.tile([P, 2, DN, P], fp16, name="wt", tag="wt")
                nc.sync.dma_start(wt[:, 0], w13_d[hi, :, 0])
                nc.sync.dma_start(wt[:, 1], w13_d[hi, :, 1])
            for off, sz in chunks:
                ph1 = ps1.tile([P, sz], f32, name="ph1", tag="ph1")
                for d in range(DN):
                    nc.tensor.matmul(
                        ph1[:],
                        wt[:, 0, d],
                        xt_sb[:, d, off : off + sz],
                        start=(d == 0),
                        stop=(d == DN - 1),
                    )
                ph3 = ps3.tile([P, sz], f32, name="ph3", tag="ph3")
                for d in range(DN):
                    nc.tensor.matmul(
                        ph3[:],
                        wt[:, 1, d],
                        xt_sb[:, d, off : off + sz],
                        start=(d == 0),
                        stop=(d == DN - 1),
                    )
                sig = silp.tile([P, sz], f32, name="sig", tag="sig")
                nc.scalar.activation(
                    sig[:], ph1[:], mybir.ActivationFunctionType.Sigmoid
                )
                sil = silp.tile([P, sz], f32, name="sil", tag="sil")
                nc.vector.tensor_mul(sil[:], sig[:], ph1[:])
                nc.vector.tensor_mul(ht_sb[:, hi, off : off + sz], sil[:], ph3[:])

        # gates are only needed by phase 2; load late so the startup DMAs
        # (first weight slab + xT) get the full HBM bandwidth.
        g_sb = const.tile([P, C], f32, name="g_sb")
        nc.gpsimd.dma_start(g_sb[:], g_d[:])

        # phase 2: yT = W2T.T @ hT (contraction over H in 22 matmuls), * gate
        for dd in range(DN):
            w2t = w2p.tile([P, HN, P], fp16, name="w2t", tag="w2t")
            nc.sync.dma_start(w2t[:], w2_d[dd])
            for off, sz in chunks:
                py = psy.tile([P, sz], f32, name="py", tag="py")
                for hi in range(HN):
                    nc.tensor.matmul(
                        py[:],
                        w2t[:, hi],
                        ht_sb[:, hi, off : off + sz],
                        start=(hi == 0),
                        stop=(hi == HN - 1),
                    )
                yo = outp.tile([P, sz], f32, name="yo", tag="yo")
                nc.vector.tensor_mul(yo[:], py[:], g_sb[:, off : off + sz])
                # outputs ride the scalar DGE queue (idle in phase 2) so they
                # don't queue behind the w2 slab loads on sync
                nc.scalar.dma_start(yt_d[dd, :, off : off + sz], yo[:])

    nc.compile()
    return nc


def _get_compiled(C):
    if C not in _COMPILED:
        _COMPILED[C] = build_bass(C)
    return _COMPILED[C]


def route(xf, router_w):
    """Host router: top-2 indices, top-2 softmax probs, aux loss (fp32 math)."""
    T = xf.shape[0]
    logits = xf @ router_w.T.astype(np.float32)
    ar = np.arange(T)
    i1 = logits.argmax(1)
    masked = logits.copy()
    masked[ar, i1] = -np.inf
    i2 = masked.argmax(1)
    l1 = logits[ar, i1]
    l2 = logits[ar, i2]
    d21 = np.exp(l2 - l1)  # <= 1
    p1 = 1.0 / (1.0 + d21)
    p2 = d21 / (1.0 + d21)

    m = logits.max(1, keepdims=True)
    ex = np.exp(logits - m)
    sumex = ex.sum(1, keepdims=True)
    all_probs = ex / sumex
    lse = m[:, 0] + np.log(sumex[:, 0])
    counts = np.bincount(np.concatenate([i1, i2]), minlength=E).astype(np.float64)
    f = counts / float(T * TOPK)
    pmean = all_probs.astype(np.float64).mean(0)
    aux = np.float32(
        LB_COEF * E * np.sum(f * pmean)
        + Z_COEF * np.mean(lse.astype(np.float64) ** 2)
    )
    return i1, i2, p1, p2, aux


def make_core_inputs(xf, w1, w3, w2, idxs, gates, C):
    in_maps = []
    for e in range(E):
        idx = idxs[e]
        n = len(idx)
        xe = np.zeros((C, D), np.float16)
        xe[:n] = xf[idx]
        xt = np.ascontiguousarray(xe.T.reshape(DN, P, C).transpose(1, 0, 2))

        w13 = np.stack([w1[e], w3[e]])          # [2, H, D] = [w, hi*128+m, d*128+k]
        w13 = w13.reshape(2, HN, P, DN, P)      # [w, hi, m, d, k]
        w13 = np.ascontiguousarray(
            w13.transpose(1, 4, 0, 3, 2), dtype=np.float16
        )                                        # [hi, k, w, d, m]

        w2t = w2[e].reshape(DN, P, HN, P)       # [dd, m, hi, k]
        w2t = np.ascontiguousarray(
            w2t.transpose(0, 3, 2, 1), dtype=np.float16
        )                                        # [dd, k, hi, m]

        gb = np.zeros((C,), np.float32)
        gb[:n] = gates[e]
        g2 = np.ascontiguousarray(np.broadcast_to(gb, (P, C)))

        in_maps.append({"xt": xt, "w13": w13, "w2t": w2t, "g": g2})
    return in_maps


def kernel(x, router_w, w1, w3, w2):
    x = np.asarray(x, dtype=np.float32)
    router_w = np.asarray(router_w, dtype=np.float32)
    w1 = np.asarray(w1, dtype=np.float32)
    w3 = np.asarray(w3, dtype=np.float32)
    w2 = np.asarray(w2, dtype=np.float32)

    B, S, _ = x.shape
    T = B * S
    xf = x.reshape(T, D)

    i1, i2, p1, p2, aux = route(xf, router_w)

    idxs, gates = [], []
    for e in range(E):
        sel1 = i1 == e
        idx = np.nonzero(sel1 | (i2 == e))[0]
        idxs.append(idx)
        gates.append(np.where(sel1, p1, p2)[idx])
    C = capacity(max(len(ix) for ix in idxs))

    in_maps = make_core_inputs(xf, w1, w3, w2, idxs, gates, C)

    from concourse.bass_utils import run_bass_kernel_spmd

    nc = _get_compiled(C)
    res = run_bass_kernel_spmd(nc, in_maps, core_ids=list(range(NCORES)))

    y = np.zeros((T, D), np.float32)
    for e in range(E):
        yt = res.results[e]["yt"]               # [DN, P, C] f32
        ye = np.asarray(yt, np.float32).reshape(D, C).T
        idx = idxs[e]
        y[idx] += ye[: len(idx)]
    return y.reshape(B, S, D), aux
